# revision 4
# baseline (speedup 1.0000x reference)
"""Trainium2 Bass kernel for NeuralTensorLayer (order-1/2/3 polynomial layer).

    out[b,l] = bias[l] + sum_i X[b,i] W1[i,l]
             + sum_ij X[b,i] X[b,j] W2[i,j,l]
             + sum_ijk X[b,i] X[b,j] X[b,k] W3[i,j,k,l]

with B=32768, D=K=32, data-parallel over 8 NeuronCores (4096 rows each).

Strategy (per core):
  * Full order-3 symmetrization: monomial {a<=b<=c} is charged once, to pair
    (a,b) at k-column c, with weight = sum of W3 over distinct permutations.
    Pairs sorted by descending j and grouped into 6 chunks with j-ranges
    [31..28],[27..24],[23..20],[19..15],[14..9],[8..0]+ones; chunk with
    j >= jlo only streams k-columns k >= jlo (even-padded JLO), cutting PE
    moving columns from 5x1088 to ~3328 per 128-row tile.
  * PSUM layout is l-major, bank-aligned: col l*32+k (two 512-col banks of
    16 l-blocks each) plus a 32-col out_low block (W2 pairs + bias on the
    virtual ones row; W1 rides the ones row's k-grid).  The narrowest chunk
    goes first with start=True (clears banks; untouched columns overwrite on
    first touch via has_written), the full jlo=0 chunk goes last with
    stop=True so every column's final writer carries the stop flag.
  * Pair operands arrive host-pregathered, packed [128, 6, B] bf16; two DMAs
    per half-supertile on parallel rings (XE via sync HWDGE, XR via gpsimd
    SWDGE) so the first Z build starts ~2us after launch.
  * Post per tile: ScalarE stages the k-grid to SBUF bf16 and casts out_low
    to f32; DVE multiplies by x (broadcast over l, 2x bf16), reduces over k,
    and adds out_low.  One batched output DMA per supertile.
  * Redundant LDWEIGHTS are stripped from the BIR before codegen (matmuls
    sharing a stationary operand across the three column-range splits).
"""

import numpy as np
import ml_dtypes
from contextlib import ExitStack

import concourse.bass as bass
import concourse.bacc as bacc
import concourse.tile as tile
from concourse import mybir
from concourse import bass_utils

BF16 = ml_dtypes.bfloat16

B, D, KOUT = 32768, 32, 32
NCORES = 8
BLOC = B // NCORES          # 4096 rows per core
P = 128                     # rows per tile
SUPER = 4                   # tiles per supertile
SP = SUPER * P              # 512
NSUPER = BLOC // SP         # 8
NCHUNK = 6
CH = [122, 106, 90, 90, 75, 46]     # rows per chunk (pairs by desc j, +ones)
JLO = [28, 24, 20, 14, 8, 0]        # lowest streamed k per chunk (even)
KW = [32 - j for j in JLO]          # streamed k-columns per chunk
NGRID = KOUT * D            # 1024 k-grid psum cols, col = l*32+k
NCOL = NGRID + KOUT         # + 32 out_low cols
WOFF = np.cumsum([0] + [32 * kw + KOUT for kw in KW]).tolist()

# pair order: descending j, ascending i; ones row last (chunk 5)
ROWS = [(i, j) for j in range(D - 1, -1, -1) for i in range(j + 1)]
STARTS = np.cumsum([0] + CH).tolist()

F32 = mybir.dt.float32
BF = mybir.dt.bfloat16


# Drop redundant LDWEIGHTS from the BIR before walrus codegen: matmuls that
# share a stationary operand (the three column-splits per contraction chunk)
# each carry their own Ldweights (walrus's ldw-opt pass is disabled/broken).
# A load is elided when the previous PE weight-op in SCHEDULED order has a
# byte-identical weight AP and the load itself carries no semaphore
# waits/updates (so the PE weight registers provably still hold the same
# data and no sync edge is lost).
def _dedup_ldweights(bir_json: bytes) -> bytes:
    import json as _json

    d = _json.loads(bir_json)
    for fn in d.get("functions", []):
        for blk in fn.get("blocks", []):
            out = []
            last = None
            for i in blk.get("instructions", []):
                if i.get("engine") == "PE" and i.get("opcode") in ("Ldweights", "Matmult"):
                    w = i["ins"][-1] if i["opcode"] == "Matmult" else i["ins"][0]
                    key = (w.get("memref"), w.get("offset"), _json.dumps(w.get("ap")),
                           w.get("dtype"), _json.dumps(i.get("tile_position")),
                           _json.dumps(i.get("tile_size")), i.get("perf_mode"))
                    if i["opcode"] == "Ldweights":
                        si = i.get("sync_info") or {}
                        if (key == last and not si.get("on_wait")
                                and not si.get("on_update")):
                            continue
                        last = key
                    else:
                        last = key
                elif i.get("engine") == "PE":
                    last = None  # unknown PE op: invalidate weight-reuse state
                out.append(i)
            blk["instructions"] = out
    return _json.dumps(d).encode()


if not getattr(bass_utils, "_ldw_dedup_patched", False):
    _orig_compile_bir_kernel = bass_utils.compile_bir_kernel

    def _compile_bir_kernel_dedup(bir_json, tmpdir, neff_name="file.neff"):
        return _orig_compile_bir_kernel(_dedup_ldweights(bir_json), tmpdir, neff_name)

    bass_utils.compile_bir_kernel = _compile_bir_kernel_dedup
    import concourse.bass2jax as _b2j

    _b2j.compile_bir_kernel = _compile_bir_kernel_dedup
    bass_utils._ldw_dedup_patched = True


def _pack_weights(W1, W2, W3, bias):
    W1 = np.asarray(W1, np.float64)
    W2 = np.asarray(W2, np.float64)
    W3 = np.asarray(W3, np.float64)
    bias = np.asarray(bias, np.float64).reshape(KOUT)
    # fully symmetrized order-3: wsym[a,b,c] = sum over distinct perms, for
    # a<=b<=c; charged to pair (a,b) at k-column c.
    W3s = W3 + W3.transpose(1, 0, 2, 3)          # i<->j symmetrized [i,j,k,l]
    Wv = np.zeros((128, WOFF[-1]), np.float64)
    for c in range(NCHUNK):
        jlo, kw = JLO[c], KW[c]
        base = WOFF[c]
        rows = ROWS[STARTS[c]:STARTS[c + 1]]
        for loc, (i, j) in enumerate(rows):
            # k-grid block: [h 2][l16 16][k kw], value for k = jlo+kk
            blk = np.zeros((KOUT, kw), np.float64)   # [l, kk]
            for kk in range(kw):
                k = jlo + kk
                if k < j:
                    continue
                if i == j == k:
                    w = W3[i, i, i]
                elif i == j:
                    w = W3[i, i, k] + W3[i, k, i] + W3[k, i, i]
                elif j == k:
                    w = W3s[i, j, j] + W3[j, j, i]      # (i,j,j),(j,i,j),(j,j,i)
                else:
                    w = (W3[i, j, k] + W3[j, i, k] + W3[i, k, j]
                         + W3[k, i, j] + W3[j, k, i] + W3[k, j, i])
                blk[:, kk] = w
            Wv[loc, base:base + 32 * kw] = blk.reshape(
                2, 16, kw).reshape(2, 16 * kw).reshape(-1)
            w2 = W2[i, j] + W2[j, i] if i < j else W2[i, i]
            Wv[loc, base + 32 * kw:base + 32 * kw + KOUT] = w2
        if c == NCHUNK - 1:
            loc = len(rows)   # ones row
            blk = np.zeros((KOUT, kw), np.float64)
            for kk in range(kw):
                blk[:, kk] = W1[jlo + kk]
            Wv[loc, base:base + 32 * kw] = blk.reshape(2, 16 * kw).reshape(-1)
            Wv[loc, base + 32 * kw:base + 32 * kw + KOUT] = bias
    return Wv.astype(np.float32).astype(BF16)


def _build_module():
    nc = bacc.Bacc("TRN2", target_bir_lowering=False, debug=False,
                   enable_asserts=False)
    XBd = nc.dram_tensor("XB", [128, NSUPER * SUPER * D], BF,
                         kind="ExternalInput").ap()
    XEd = nc.dram_tensor("XE", [128, NCHUNK, BLOC], BF, kind="ExternalInput").ap()
    XRd = nc.dram_tensor("XR", [128, NCHUNK, BLOC], BF, kind="ExternalInput").ap()
    WCd = nc.dram_tensor("WCAT", [128, WOFF[-1]], BF, kind="ExternalInput").ap()
    OUTd = nc.dram_tensor("OUT", [BLOC, KOUT], F32, kind="ExternalOutput").ap()

    with ExitStack() as ctx:
        tc = ctx.enter_context(tile.TileContext(nc))
        consts = ctx.enter_context(tc.tile_pool(name="consts", bufs=1))
        xepool = ctx.enter_context(tc.tile_pool(name="xepool", bufs=2))
        zpool = ctx.enter_context(tc.tile_pool(name="zpool", bufs=2))
        spool = ctx.enter_context(tc.tile_pool(name="spool", bufs=3))
        upool = ctx.enter_context(tc.tile_pool(name="upool", bufs=3))
        rpool = ctx.enter_context(tc.tile_pool(name="rpool", bufs=3))
        opool = ctx.enter_context(tc.tile_pool(name="opool", bufs=2))
        t3ps = ctx.enter_context(tc.tile_pool(name="t3ps", bufs=2, space="PSUM"))

        wv = consts.tile([128, WOFF[-1]], BF, tag="wv")
        nc.scalar.dma_start(out=wv, in_=WCd)
        xball = consts.tile([128, NSUPER * SUPER * D], BF, tag="xball")
        nc.scalar.dma_start(out=xball, in_=XBd)

        def fetch(s):
            xa = xepool.tile([128, 3 * SP], BF, tag="xa")
            nc.sync.dma_start(out=xa, in_=XEd[:, 0:3, s * SP:(s + 1) * SP])
            xb = xepool.tile([128, 3 * SP], BF, tag="xb")
            nc.sync.dma_start(out=xb, in_=XEd[:, 3:6, s * SP:(s + 1) * SP])
            ra = xepool.tile([128, 3 * SP], BF, tag="ra")
            nc.gpsimd.dma_start(out=ra, in_=XRd[:, 0:3, s * SP:(s + 1) * SP])
            rb = xepool.tile([128, 3 * SP], BF, tag="rb")
            nc.gpsimd.dma_start(out=rb, in_=XRd[:, 3:6, s * SP:(s + 1) * SP])
            return xa, xb, ra, rb

        # chunk issue order: narrowest (clears banks) ... full jlo=0 (stop)
        fet = fetch(0)
        for s in range(NSUPER):
            xa, xb, ra, rb = fet
            z = zpool.tile([128, NCHUNK * SP], BF, tag="z")
            nc.vector.tensor_mul(z[:, 0:3 * SP], xa, ra)
            nc.vector.tensor_mul(z[:, 3 * SP:6 * SP], xb, rb)
            if s + 1 < NSUPER:
                fet = fetch(s + 1)
            obuf = opool.tile([128, SUPER * KOUT], F32, tag="obuf")
            for t in range(SUPER):
                t3 = t3ps.tile([P, NCOL], F32, tag="t3")
                for c in range(NCHUNK):
                    kw, jlo = KW[c], JLO[c]
                    first, last = c == 0, c == NCHUNK - 1
                    zc = z[:CH[c], c * SP + t * P: c * SP + (t + 1) * P]
                    for h in range(2):
                        nc.tensor.matmul(
                            t3[:, h * 512:(h + 1) * 512].rearrange(
                                "p (l k) -> p l k", k=D)[:, :, jlo:],
                            zc,
                            wv[:CH[c], WOFF[c] + h * 16 * kw:
                               WOFF[c] + (h + 1) * 16 * kw].rearrange(
                                "p (l k) -> p l k", k=kw),
                            start=first, stop=last)
                    nc.tensor.matmul(
                        t3[:, NGRID:NCOL],
                        zc,
                        wv[:CH[c], WOFF[c] + 32 * kw: WOFF[c] + 32 * kw + KOUT],
                        start=first, stop=last)
                staged = spool.tile([P, NGRID], BF, tag="staged")
                nc.scalar.copy(out=staged, in_=t3[:, :NGRID])
                olf = rpool.tile([P, KOUT], F32, tag="olf")
                nc.scalar.copy(out=olf, in_=t3[:, NGRID:NCOL])
                u = upool.tile([P, NGRID], BF, tag="u")
                off = (s * SUPER + t) * D
                xk = xball[:, off:off + D].unsqueeze(1).broadcast_to(
                    [P, KOUT, D])
                nc.vector.tensor_mul(
                    u[:, :].rearrange("p (l k) -> p l k", k=D),
                    staged[:, :].rearrange("p (l k) -> p l k", k=D),
                    xk,
                )
                rtmp = rpool.tile([P, KOUT], F32, tag="rtmp")
                nc.vector.reduce_sum(
                    out=rtmp,
                    in_=u[:, :].rearrange("p (l k) -> p l k", k=D),
                    axis=mybir.AxisListType.X,
                )
                nc.vector.tensor_add(obuf[:, t * KOUT:(t + 1) * KOUT],
                                     rtmp, olf)
            nc.scalar.dma_start(
                out=OUTd[s * SP:(s + 1) * SP, :].rearrange(
                    "(t p) l -> p t l", t=SUPER),
                in_=obuf[:, :].rearrange("p (t l) -> p t l", l=KOUT),
            )
    nc.compile()
    return nc


_CACHE = {}


def _get_module():
    if "nc" not in _CACHE:
        _CACHE["nc"] = _build_module()
    return _CACHE["nc"]


def kernel(X, W1, W2, W3, bias):
    X = np.ascontiguousarray(np.asarray(X, np.float32))
    Wcat = _pack_weights(W1, W2, W3, bias)

    nc = _get_module()
    Xb = X.astype(BF16)                      # [B, D] bf16 (single rounding point)
    XbT = np.ascontiguousarray(Xb.T)         # [D, B] bf16
    XE = np.zeros((NCHUNK, 128, B), BF16)
    XR = np.zeros((NCHUNK, 128, B), BF16)
    for c in range(NCHUNK):
        rows = ROWS[STARTS[c]:STARTS[c + 1]]
        for loc, (i, j) in enumerate(rows):
            XE[c, loc] = XbT[i]
            XR[c, loc] = XbT[j]
        if c == NCHUNK - 1:
            XE[c, len(rows)] = BF16(1.0)
            XR[c, len(rows)] = BF16(1.0)
    # packed layouts: [core][part 128][chunk 6][bloc]
    XEp = XE.reshape(NCHUNK, 128, NCORES, BLOC).transpose(2, 1, 0, 3)
    XRp = XR.reshape(NCHUNK, 128, NCORES, BLOC).transpose(2, 1, 0, 3)
    # [core][part 128][supertile*tile][d]
    XBp = Xb.reshape(NCORES, NSUPER, SUPER, P, D).transpose(
        0, 3, 1, 2, 4).reshape(NCORES, P, NSUPER * SUPER * D)
    in_maps = [
        {
            "XB": np.ascontiguousarray(XBp[c]),
            "XE": np.ascontiguousarray(XEp[c]),
            "XR": np.ascontiguousarray(XRp[c]),
            "WCAT": Wcat,
        }
        for c in range(NCORES)
    ]
    res = bass_utils.run_bass_kernel_spmd(nc, in_maps, core_ids=list(range(NCORES)))
    _CACHE["last_results"] = res
    out = np.concatenate([np.asarray(res.results[c]["OUT"]) for c in range(NCORES)], 0)
    return out.astype(np.float32)


# revision 5
# speedup vs baseline: 1.2184x; 1.2184x over previous
"""Trainium2 Bass kernel for NeuralTensorLayer (order-1/2/3 polynomial layer).

    out[b,l] = bias[l] + sum_i X[b,i] W1[i,l]
             + sum_ij X[b,i] X[b,j] W2[i,j,l]
             + sum_ijk X[b,i] X[b,j] X[b,k] W3[i,j,k,l]

with B=32768, D=K=32, data-parallel over 8 NeuronCores (4096 rows each).

Strategy (per core):
  * (i,j) symmetry: 528 pairs i<=j against host-symmetrized weights, plus a
    single virtual "ones" contraction row (x_32 == 1) that carries W1 on the
    k-grid and bias on the out_low column -> 529 contraction rows in chunks
    of [128,128,128,128,17].
  * PSUM layout [128, 1056] f32: l-major k-grid col l*32+k in two bank-
    aligned 512-col halves, plus a contiguous 32-col out_low block (W2
    pairs + bias).  All matmul outputs are contiguous 2D APs (strided PSUM
    matmul writes pay a ~2-4x per-segment penalty on TRN2).
  * Pair operands arrive host-pregathered, packed [128, 5, B] bf16, fetched
    on two parallel DMA rings (XE via sync HWDGE, XR via gpsimd SWDGE) in
    two half-supertile pieces so the first Z build starts early; the DVE
    builds Z^T = XE*XR in two multiplies (2x bf16 mode).
  * Post per tile: ScalarE stages the k-grid to SBUF bf16 and casts out_low
    to f32; DVE multiplies by x (broadcast over l, 2x bf16), reduces over
    k=32, and adds out_low.  One batched output DMA per supertile.
  * Redundant LDWEIGHTS are stripped from the BIR before codegen (matmuls
    sharing a stationary operand across the three column-splits).
"""

import numpy as np
import ml_dtypes
from contextlib import ExitStack

import concourse.bass as bass
import concourse.bacc as bacc
import concourse.tile as tile
from concourse import mybir
from concourse import bass_utils

BF16 = ml_dtypes.bfloat16

B, D, KOUT = 32768, 32, 32
NCORES = 8
BLOC = B // NCORES          # 4096 rows per core
P = 128                     # rows per tile
SUPER = 4                   # tiles per supertile
SP = SUPER * P              # 512
NSUPER = BLOC // SP         # 8
NPAIRS = D * (D + 1) // 2   # 528
NROWS = NPAIRS + 1          # + ones row
NCHUNK = 5
CHUNK_P = [128, 128, 128, 128, 17]  # partitions per contraction chunk
NGRID = KOUT * D            # 1024 k-grid psum cols, col = l*32+k
NCOL = NGRID + KOUT         # + 32 out_low cols

PAIRS = [(i, j) for i in range(D) for j in range(i, D)]
I_P = np.array([p[0] for p in PAIRS], np.int32)
J_P = np.array([p[1] for p in PAIRS], np.int32)

F32 = mybir.dt.float32
BF = mybir.dt.bfloat16


# Drop redundant LDWEIGHTS from the BIR before walrus codegen: matmuls that
# share a stationary operand (the three column-splits per contraction chunk)
# each carry their own Ldweights (walrus's ldw-opt pass is disabled/broken).
# A load is elided when the previous PE weight-op in SCHEDULED order has a
# byte-identical weight AP and the load itself carries no semaphore
# waits/updates (so the PE weight registers provably still hold the same
# data and no sync edge is lost).
def _dedup_ldweights(bir_json: bytes) -> bytes:
    import json as _json

    d = _json.loads(bir_json)
    for fn in d.get("functions", []):
        for blk in fn.get("blocks", []):
            out = []
            last = None
            for i in blk.get("instructions", []):
                if i.get("engine") == "PE" and i.get("opcode") in ("Ldweights", "Matmult"):
                    w = i["ins"][-1] if i["opcode"] == "Matmult" else i["ins"][0]
                    key = (w.get("memref"), w.get("offset"), _json.dumps(w.get("ap")),
                           w.get("dtype"), _json.dumps(i.get("tile_position")),
                           _json.dumps(i.get("tile_size")), i.get("perf_mode"))
                    if i["opcode"] == "Ldweights":
                        si = i.get("sync_info") or {}
                        if (key == last and not si.get("on_wait")
                                and not si.get("on_update")):
                            continue
                        last = key
                    else:
                        last = key
                elif i.get("engine") == "PE":
                    last = None  # unknown PE op: invalidate weight-reuse state
                out.append(i)
            blk["instructions"] = out
    return _json.dumps(d).encode()


if not getattr(bass_utils, "_ldw_dedup_patched", False):
    _orig_compile_bir_kernel = bass_utils.compile_bir_kernel

    def _compile_bir_kernel_dedup(bir_json, tmpdir, neff_name="file.neff"):
        return _orig_compile_bir_kernel(_dedup_ldweights(bir_json), tmpdir, neff_name)

    bass_utils.compile_bir_kernel = _compile_bir_kernel_dedup
    import concourse.bass2jax as _b2j

    _b2j.compile_bir_kernel = _compile_bir_kernel_dedup
    bass_utils._ldw_dedup_patched = True


def _pack_weights(W1, W2, W3, bias):
    W1 = np.asarray(W1, np.float64)
    W2 = np.asarray(W2, np.float64)
    W3 = np.asarray(W3, np.float64)
    bias = np.asarray(bias, np.float64).reshape(KOUT)
    Wcat = np.zeros((NCHUNK, 128, NCOL), np.float64)
    for p, (i, j) in enumerate(PAIRS):
        c, pp = divmod(p, 128)
        if i < j:
            w3 = W3[i, j] + W3[j, i]   # [k, l]
            w2 = W2[i, j] + W2[j, i]   # [l]
        else:
            w3 = W3[i, i]
            w2 = W2[i, i]
        Wcat[c, pp, :NGRID] = w3.T.reshape(-1)     # col l*32+k
        Wcat[c, pp, NGRID:] = w2                   # out_low block
    c, pp = divmod(NPAIRS, 128)                    # ones row
    Wcat[c, pp, :NGRID] = W1.T.reshape(-1)         # col l*32+k = W1[k, l]
    Wcat[c, pp, NGRID:] = bias
    return Wcat.reshape(NCHUNK * 128, NCOL).astype(np.float32).astype(BF16)


def _build_module():
    nc = bacc.Bacc("TRN2", target_bir_lowering=False, debug=False,
                   enable_asserts=False)
    XBd = nc.dram_tensor("XB", [128, NSUPER * SUPER * D], BF,
                         kind="ExternalInput").ap()
    XEd = nc.dram_tensor("XE", [128, NCHUNK, BLOC], BF, kind="ExternalInput").ap()
    XRd = nc.dram_tensor("XR", [128, NCHUNK, BLOC], BF, kind="ExternalInput").ap()
    WCd = nc.dram_tensor("WCAT", [NCHUNK, 128, NCOL], BF,
                         kind="ExternalInput").ap()
    OUTd = nc.dram_tensor("OUT", [BLOC, KOUT], F32, kind="ExternalOutput").ap()

    with ExitStack() as ctx:
        tc = ctx.enter_context(tile.TileContext(nc))
        consts = ctx.enter_context(tc.tile_pool(name="consts", bufs=1))
        xepool = ctx.enter_context(tc.tile_pool(name="xepool", bufs=2))
        zpool = ctx.enter_context(tc.tile_pool(name="zpool", bufs=2))
        spool = ctx.enter_context(tc.tile_pool(name="spool", bufs=3))
        upool = ctx.enter_context(tc.tile_pool(name="upool", bufs=3))
        rpool = ctx.enter_context(tc.tile_pool(name="rpool", bufs=3))
        opool = ctx.enter_context(tc.tile_pool(name="opool", bufs=2))
        t3ps = ctx.enter_context(tc.tile_pool(name="t3ps", bufs=2, space="PSUM"))

        w_sb = []
        for c in range(NCHUNK):
            w = consts.tile([128, NCOL], BF, tag=f"w_{c}")
            nc.scalar.dma_start(out=w, in_=WCd[c])
            w_sb.append(w)
        xball = consts.tile([128, NSUPER * SUPER * D], BF, tag="xball")
        nc.scalar.dma_start(out=xball, in_=XBd)

        def fetch(s):
            xa = xepool.tile([128, 2 * SP], BF, tag="xa")
            nc.sync.dma_start(out=xa, in_=XEd[:, 0:2, s * SP:(s + 1) * SP])
            xb = xepool.tile([128, 3 * SP], BF, tag="xb")
            nc.sync.dma_start(out=xb, in_=XEd[:, 2:5, s * SP:(s + 1) * SP])
            ra = xepool.tile([128, 2 * SP], BF, tag="ra")
            nc.gpsimd.dma_start(out=ra, in_=XRd[:, 0:2, s * SP:(s + 1) * SP])
            rb = xepool.tile([128, 3 * SP], BF, tag="rb")
            nc.gpsimd.dma_start(out=rb, in_=XRd[:, 2:5, s * SP:(s + 1) * SP])
            return xa, xb, ra, rb

        fet = fetch(0)
        for s in range(NSUPER):
            xa, xb, ra, rb = fet
            if s + 1 < NSUPER:
                fet = fetch(s + 1)
            z = zpool.tile([128, NCHUNK * SP], BF, tag="z")
            nc.vector.tensor_mul(z[:, 0:2 * SP], xa, ra)
            nc.vector.tensor_mul(z[:, 2 * SP:5 * SP], xb, rb)
            obuf = opool.tile([128, SUPER * KOUT], F32, tag="obuf")
            for t in range(SUPER):
                t3 = t3ps.tile([P, NCOL], F32, tag="t3")
                for c in range(NCHUNK):
                    pcp = CHUNK_P[c]
                    first, last = c == 0, c == NCHUNK - 1
                    zc = z[:pcp, c * SP + t * P: c * SP + (t + 1) * P]
                    for n0, n1 in ((0, 512), (512, 1024), (1024, NCOL)):
                        nc.tensor.matmul(t3[:, n0:n1], zc,
                                         w_sb[c][:pcp, n0:n1],
                                         start=first, stop=last)
                staged = spool.tile([P, NGRID], BF, tag="staged")
                nc.scalar.copy(out=staged, in_=t3[:, :NGRID])
                olf = rpool.tile([P, KOUT], F32, tag="olf")
                nc.scalar.copy(out=olf, in_=t3[:, NGRID:NCOL])
                u = upool.tile([P, NGRID], BF, tag="u")
                off = (s * SUPER + t) * D
                xk = xball[:, off:off + D].unsqueeze(1).broadcast_to(
                    [P, KOUT, D])
                nc.vector.tensor_mul(
                    u[:, :].rearrange("p (l k) -> p l k", k=D),
                    staged[:, :].rearrange("p (l k) -> p l k", k=D),
                    xk,
                )
                rtmp = rpool.tile([P, KOUT], F32, tag="rtmp")
                nc.vector.reduce_sum(
                    out=rtmp,
                    in_=u[:, :].rearrange("p (l k) -> p l k", k=D),
                    axis=mybir.AxisListType.X,
                )
                nc.vector.tensor_add(obuf[:, t * KOUT:(t + 1) * KOUT],
                                     rtmp, olf)
            nc.scalar.dma_start(
                out=OUTd[s * SP:(s + 1) * SP, :].rearrange(
                    "(t p) l -> p t l", t=SUPER),
                in_=obuf[:, :].rearrange("p (t l) -> p t l", l=KOUT),
            )
    nc.compile()
    return nc


_CACHE = {}


def _get_module():
    if "nc" not in _CACHE:
        _CACHE["nc"] = _build_module()
    return _CACHE["nc"]


def kernel(X, W1, W2, W3, bias):
    X = np.ascontiguousarray(np.asarray(X, np.float32))
    Wcat = _pack_weights(W1, W2, W3, bias).reshape(NCHUNK, 128, NCOL)

    nc = _get_module()
    Xb = X.astype(BF16)                      # [B, D] bf16 (single rounding point)
    XbT = np.ascontiguousarray(Xb.T)         # [D, B] bf16
    npad = NCHUNK * 128 - NROWS
    ones_row = np.ones((1, B), BF16)
    zpad = np.zeros((npad, B), BF16)
    XE = np.concatenate([XbT[I_P], ones_row, zpad], 0).reshape(NCHUNK, 128, B)
    XR = np.concatenate([XbT[J_P], ones_row, zpad], 0).reshape(NCHUNK, 128, B)
    # packed layouts: [core][part 128][chunk 5][bloc]
    XEp = XE.reshape(NCHUNK, 128, NCORES, BLOC).transpose(2, 1, 0, 3)
    XRp = XR.reshape(NCHUNK, 128, NCORES, BLOC).transpose(2, 1, 0, 3)
    # [core][part 128][supertile*tile][d]
    XBp = Xb.reshape(NCORES, NSUPER, SUPER, P, D).transpose(
        0, 3, 1, 2, 4).reshape(NCORES, P, NSUPER * SUPER * D)
    in_maps = [
        {
            "XB": np.ascontiguousarray(XBp[c]),
            "XE": np.ascontiguousarray(XEp[c]),
            "XR": np.ascontiguousarray(XRp[c]),
            "WCAT": Wcat,
        }
        for c in range(NCORES)
    ]
    res = bass_utils.run_bass_kernel_spmd(nc, in_maps, core_ids=list(range(NCORES)))
    _CACHE["last_results"] = res
    out = np.concatenate([np.asarray(res.results[c]["OUT"]) for c in range(NCORES)], 0)
    return out.astype(np.float32)


# revision 7
# speedup vs baseline: 1.2318x; 1.0110x over previous
"""Trainium2 Bass kernel for NeuralTensorLayer (order-1/2/3 polynomial layer).

    out[b,l] = bias[l] + sum_i X[b,i] W1[i,l]
             + sum_ij X[b,i] X[b,j] W2[i,j,l]
             + sum_ijk X[b,i] X[b,j] X[b,k] W3[i,j,k,l]

with B=32768, D=K=32, data-parallel over 8 NeuronCores (4096 rows each).

Strategy (per core):
  * (i,j) symmetry: 528 pairs i<=j against host-symmetrized weights, plus a
    single virtual "ones" contraction row (x_32 == 1) that carries W1 on the
    k-grid and bias on the out_low column -> 529 contraction rows in chunks
    of [128,128,128,128,17].
  * PSUM layout [128, 1056] f32: l-major k-grid col l*32+k in two bank-
    aligned 512-col halves, plus a contiguous 32-col out_low block (W2
    pairs + bias).  All matmul outputs are contiguous 2D APs (strided PSUM
    matmul writes pay a ~2-4x per-segment penalty on TRN2).
  * Pair operands arrive host-pregathered, packed [128, 5, B] bf16, fetched
    on two parallel DMA rings (XE via sync HWDGE, XR via gpsimd SWDGE) in
    two half-supertile pieces so the first Z build starts early; the DVE
    builds Z^T = XE*XR in two multiplies (2x bf16 mode).
  * Post per tile: ScalarE stages the k-grid to SBUF bf16 and casts out_low
    to f32; DVE multiplies by x (broadcast over l, 2x bf16), reduces over
    k=32, and adds out_low.  One batched output DMA per supertile.
  * Redundant LDWEIGHTS are stripped from the BIR before codegen (matmuls
    sharing a stationary operand across the three column-splits).
"""

import numpy as np
import ml_dtypes
from contextlib import ExitStack

import concourse.bass as bass
import concourse.bacc as bacc
import concourse.tile as tile
from concourse import mybir
from concourse import bass_utils

BF16 = ml_dtypes.bfloat16

B, D, KOUT = 32768, 32, 32
NCORES = 8
BLOC = B // NCORES          # 4096 rows per core
P = 128                     # rows per tile
SUPER = 4                   # tiles per supertile
SP = SUPER * P              # 512
NSUPER = BLOC // SP         # 8
NPAIRS = D * (D + 1) // 2   # 528
NROWS = NPAIRS + 1          # + ones row
NCHUNK = 5
CHUNK_P = [128, 128, 128, 128, 17]  # partitions per contraction chunk
NGRID = KOUT * D            # 1024 k-grid psum cols, col = l*32+k
NCOL = NGRID + KOUT         # + 32 out_low cols

PAIRS = [(i, j) for i in range(D) for j in range(i, D)]
I_P = np.array([p[0] for p in PAIRS], np.int32)
J_P = np.array([p[1] for p in PAIRS], np.int32)

F32 = mybir.dt.float32
BF = mybir.dt.bfloat16


# Drop redundant LDWEIGHTS from the BIR before walrus codegen: matmuls that
# share a stationary operand (the three column-splits per contraction chunk)
# each carry their own Ldweights (walrus's ldw-opt pass is disabled/broken).
# A load is elided when the previous PE weight-op in SCHEDULED order has a
# byte-identical weight AP and the load itself carries no semaphore
# waits/updates (so the PE weight registers provably still hold the same
# data and no sync edge is lost).
def _dedup_ldweights(bir_json: bytes) -> bytes:
    import json as _json

    d = _json.loads(bir_json)
    for fn in d.get("functions", []):
        for blk in fn.get("blocks", []):
            out = []
            last = None
            for i in blk.get("instructions", []):
                if i.get("engine") == "PE" and i.get("opcode") in ("Ldweights", "Matmult"):
                    w = i["ins"][-1] if i["opcode"] == "Matmult" else i["ins"][0]
                    key = (w.get("memref"), w.get("offset"), _json.dumps(w.get("ap")),
                           w.get("dtype"), _json.dumps(i.get("tile_position")),
                           _json.dumps(i.get("tile_size")), i.get("perf_mode"))
                    if i["opcode"] == "Ldweights":
                        si = i.get("sync_info") or {}
                        if (key == last and not si.get("on_wait")
                                and not si.get("on_update")):
                            continue
                        last = key
                    else:
                        last = key
                elif i.get("engine") == "PE":
                    last = None  # unknown PE op: invalidate weight-reuse state
                out.append(i)
            blk["instructions"] = out
    return _json.dumps(d).encode()


if not getattr(bass_utils, "_ldw_dedup_patched", False):
    _orig_compile_bir_kernel = bass_utils.compile_bir_kernel

    def _compile_bir_kernel_dedup(bir_json, tmpdir, neff_name="file.neff"):
        return _orig_compile_bir_kernel(_dedup_ldweights(bir_json), tmpdir, neff_name)

    bass_utils.compile_bir_kernel = _compile_bir_kernel_dedup
    import concourse.bass2jax as _b2j

    _b2j.compile_bir_kernel = _compile_bir_kernel_dedup
    bass_utils._ldw_dedup_patched = True


def _pack_weights(W1, W2, W3, bias):
    W1 = np.asarray(W1, np.float64)
    W2 = np.asarray(W2, np.float64)
    W3 = np.asarray(W3, np.float64)
    bias = np.asarray(bias, np.float64).reshape(KOUT)
    Wcat = np.zeros((NCHUNK, 128, NCOL), np.float64)
    for p, (i, j) in enumerate(PAIRS):
        c, pp = divmod(p, 128)
        if i < j:
            w3 = W3[i, j] + W3[j, i]   # [k, l]
            w2 = W2[i, j] + W2[j, i]   # [l]
        else:
            w3 = W3[i, i]
            w2 = W2[i, i]
        Wcat[c, pp, :NGRID] = w3.T.reshape(-1)     # col l*32+k
        Wcat[c, pp, NGRID:] = w2                   # out_low block
    c, pp = divmod(NPAIRS, 128)                    # ones row
    Wcat[c, pp, :NGRID] = W1.T.reshape(-1)         # col l*32+k = W1[k, l]
    Wcat[c, pp, NGRID:] = bias
    return Wcat.reshape(NCHUNK * 128, NCOL).astype(np.float32).astype(BF16)


def _build_module():
    nc = bacc.Bacc("TRN2", target_bir_lowering=False, debug=False,
                   enable_asserts=False)
    XBd = nc.dram_tensor("XB", [128, NSUPER * SUPER * D], BF,
                         kind="ExternalInput").ap()
    XEd = nc.dram_tensor("XE", [128, NCHUNK, BLOC], BF, kind="ExternalInput").ap()
    XRd = nc.dram_tensor("XR", [128, NCHUNK, BLOC], BF, kind="ExternalInput").ap()
    WCd = nc.dram_tensor("WCAT", [NCHUNK, 128, NCOL], BF,
                         kind="ExternalInput").ap()
    OUTd = nc.dram_tensor("OUT", [BLOC, KOUT], F32, kind="ExternalOutput").ap()

    with ExitStack() as ctx:
        tc = ctx.enter_context(tile.TileContext(nc))
        consts = ctx.enter_context(tc.tile_pool(name="consts", bufs=1))
        xepool = ctx.enter_context(tc.tile_pool(name="xepool", bufs=2))
        zpool = ctx.enter_context(tc.tile_pool(name="zpool", bufs=2))
        spool = ctx.enter_context(tc.tile_pool(name="spool", bufs=3))
        upool = ctx.enter_context(tc.tile_pool(name="upool", bufs=3))
        rpool = ctx.enter_context(tc.tile_pool(name="rpool", bufs=3))
        opool = ctx.enter_context(tc.tile_pool(name="opool", bufs=2))
        t3ps = ctx.enter_context(tc.tile_pool(name="t3ps", bufs=2, space="PSUM"))

        # chunk-0 weights first: the opening matmuls need them earliest
        w_sb = [None] * NCHUNK
        for c in (0, 1):
            w = consts.tile([128, NCOL], BF, tag=f"w_{c}")
            nc.scalar.dma_start(out=w, in_=WCd[c])
            w_sb[c] = w
        xball = consts.tile([128, NSUPER * SUPER * D], BF, tag="xball")
        nc.scalar.dma_start(out=xball, in_=XBd)
        for c in (2, 3, 4):
            w = consts.tile([128, NCOL], BF, tag=f"w_{c}")
            nc.scalar.dma_start(out=w, in_=WCd[c])
            w_sb[c] = w

        def fetch(s):
            xa = xepool.tile([128, 2 * SP], BF, tag="xa")
            nc.sync.dma_start(out=xa, in_=XEd[:, 0:2, s * SP:(s + 1) * SP])
            xb = xepool.tile([128, 3 * SP], BF, tag="xb")
            nc.sync.dma_start(out=xb, in_=XEd[:, 2:5, s * SP:(s + 1) * SP])
            ra = xepool.tile([128, 2 * SP], BF, tag="ra")
            nc.gpsimd.dma_start(out=ra, in_=XRd[:, 0:2, s * SP:(s + 1) * SP])
            rb = xepool.tile([128, 3 * SP], BF, tag="rb")
            nc.gpsimd.dma_start(out=rb, in_=XRd[:, 2:5, s * SP:(s + 1) * SP])
            return xa, xb, ra, rb

        # supertile 0 is fetched at tile granularity so the first matmuls
        # start ~10us earlier (the first DMA wave is bandwidth-bound) and the
        # PE warms up without early idle gaps.
        z0t = []
        for t in range(SUPER):
            xt = xepool.tile([128, NCHUNK * P], BF, tag=f"x0_{t}")
            nc.sync.dma_start(out=xt, in_=XEd[:, :, t * P:(t + 1) * P])
            rt = xepool.tile([128, NCHUNK * P], BF, tag=f"r0_{t}")
            nc.gpsimd.dma_start(out=rt, in_=XRd[:, :, t * P:(t + 1) * P])
            zt = zpool.tile([128, NCHUNK * P], BF, tag=f"z0_{t}")
            nc.vector.tensor_mul(zt, xt, rt)
            z0t.append(zt)
        fet = fetch(1)
        for s in range(NSUPER):
            if s > 0:
                xa, xb, ra, rb = fet
                if s + 1 < NSUPER:
                    fet = fetch(s + 1)
                z = zpool.tile([128, NCHUNK * SP], BF, tag="z")
                nc.vector.tensor_mul(z[:, 0:2 * SP], xa, ra)
                nc.vector.tensor_mul(z[:, 2 * SP:5 * SP], xb, rb)
            obuf = opool.tile([128, SUPER * KOUT], F32, tag="obuf")
            for t in range(SUPER):
                t3 = t3ps.tile([P, NCOL], F32, tag="t3")
                for c in range(NCHUNK):
                    pcp = CHUNK_P[c]
                    first, last = c == 0, c == NCHUNK - 1
                    if s == 0:
                        zc = z0t[t][:pcp, c * P:(c + 1) * P]
                    else:
                        zc = z[:pcp, c * SP + t * P: c * SP + (t + 1) * P]
                    for n0, n1 in ((0, 512), (512, 1024), (1024, NCOL)):
                        nc.tensor.matmul(t3[:, n0:n1], zc,
                                         w_sb[c][:pcp, n0:n1],
                                         start=first, stop=last)
                staged = spool.tile([P, NGRID], BF, tag="staged")
                nc.scalar.copy(out=staged, in_=t3[:, :NGRID])
                olf = rpool.tile([P, KOUT], F32, tag="olf")
                nc.scalar.copy(out=olf, in_=t3[:, NGRID:NCOL])
                u = upool.tile([P, NGRID], BF, tag="u")
                off = (s * SUPER + t) * D
                xk = xball[:, off:off + D].unsqueeze(1).broadcast_to(
                    [P, KOUT, D])
                nc.vector.tensor_mul(
                    u[:, :].rearrange("p (l k) -> p l k", k=D),
                    staged[:, :].rearrange("p (l k) -> p l k", k=D),
                    xk,
                )
                rtmp = rpool.tile([P, KOUT], F32, tag="rtmp")
                nc.vector.reduce_sum(
                    out=rtmp,
                    in_=u[:, :].rearrange("p (l k) -> p l k", k=D),
                    axis=mybir.AxisListType.X,
                )
                nc.vector.tensor_add(obuf[:, t * KOUT:(t + 1) * KOUT],
                                     rtmp, olf)
            nc.scalar.dma_start(
                out=OUTd[s * SP:(s + 1) * SP, :].rearrange(
                    "(t p) l -> p t l", t=SUPER),
                in_=obuf[:, :].rearrange("p (t l) -> p t l", l=KOUT),
            )
    nc.compile()
    return nc


_CACHE = {}


def _get_module():
    if "nc" not in _CACHE:
        _CACHE["nc"] = _build_module()
    return _CACHE["nc"]


def kernel(X, W1, W2, W3, bias):
    X = np.ascontiguousarray(np.asarray(X, np.float32))
    Wcat = _pack_weights(W1, W2, W3, bias).reshape(NCHUNK, 128, NCOL)

    nc = _get_module()
    Xb = X.astype(BF16)                      # [B, D] bf16 (single rounding point)
    XbT = np.ascontiguousarray(Xb.T)         # [D, B] bf16
    npad = NCHUNK * 128 - NROWS
    ones_row = np.ones((1, B), BF16)
    zpad = np.zeros((npad, B), BF16)
    XE = np.concatenate([XbT[I_P], ones_row, zpad], 0).reshape(NCHUNK, 128, B)
    XR = np.concatenate([XbT[J_P], ones_row, zpad], 0).reshape(NCHUNK, 128, B)
    # packed layouts: [core][part 128][chunk 5][bloc]
    XEp = XE.reshape(NCHUNK, 128, NCORES, BLOC).transpose(2, 1, 0, 3)
    XRp = XR.reshape(NCHUNK, 128, NCORES, BLOC).transpose(2, 1, 0, 3)
    # [core][part 128][supertile*tile][d]
    XBp = Xb.reshape(NCORES, NSUPER, SUPER, P, D).transpose(
        0, 3, 1, 2, 4).reshape(NCORES, P, NSUPER * SUPER * D)
    in_maps = [
        {
            "XB": np.ascontiguousarray(XBp[c]),
            "XE": np.ascontiguousarray(XEp[c]),
            "XR": np.ascontiguousarray(XRp[c]),
            "WCAT": Wcat,
        }
        for c in range(NCORES)
    ]
    res = bass_utils.run_bass_kernel_spmd(nc, in_maps, core_ids=list(range(NCORES)))
    _CACHE["last_results"] = res
    out = np.concatenate([np.asarray(res.results[c]["OUT"]) for c in range(NCORES)], 0)
    return out.astype(np.float32)
